# revision 67
# baseline (speedup 1.0000x reference)
"""Trainium2 Bass kernel for nn_Attention: per-head QKV attention + out-proj.

Contract: kernel(**inputs) takes FULL unsharded inputs
  x [8, 1024, 768] f32, Wqkv [12, 768, 192] f32, bqkv [12, 192] f32,
  Wo [768, 768] f32, bo [768] f32
returns FULL output [8, 1024, 768] f32.

Strategy: pure data-parallel over batch (8 batches -> 8 NeuronCores), no
collectives.  Each core computes its batch end-to-end in bf16 matmuls.

v3 changes vs v2 (HW ~236us, rel err 3.7e-3):
  - the two heads of a pair issue their scores matmuls back-to-back into
    DISJOINT PE row groups (K=64 each, tile_position (0,0)/(64,0)), so
    they stream concurrently: scores PE time halves (~20us).
  - PSUM locked to exactly 8 banks: "sc" slot 2x[128,1024] (scores + all
    transient matmul outputs) and "pv" slot 2x[65,1024] (accumulators).
    The "sc" slot is a 2-deep ring: EVERY tile allocated from it must
    complete its whole read-write lifecycle within 2 subsequent
    allocations or the in-order PE stream stalls (and a >3.4us stall
    re-throttles HAM to half clock for ~7us - the main failure mode).
  - softmax finalize: one [65,S] copy per head stages v-rows +
    denominator and releases pv psum early; ones65-selector matmuls
    broadcast each denominator row; full-width 128-partition
    reciprocal_approx_fast PSUM->SBUF (half-width or partition-shifted
    forms of this custom op corrupt results on real HW); h1's lanes
    reach base partition 0 via a partition-shifted SBUF copy; both
    normalize multiplies are then base-0 SBUF x SBUF.  bc_ps is held
    only ~2.4us, so the ring never stalls the PE into a HAM window.
  - qk projections for the next pair are four 6-MM n-halves placed in
    ALTERNATING slots (sk 0,2,4,6) so heavy and light slots interleave
    and the exp stream never starves; v piece2 fillers fall in the odd
    slots of pairs 1-2.
  - output projection split kc 0-3 / 4-5: the first half runs as
    late-loop fillers (pairs 4-5) with bias folded into bf16 partials;
    only kc 4-5 + the staged add remain after the last pair, with 16
    warmup matmuls keeping HAM warm through the finalize chain and the
    output DMA triggers spread across sync/gpsimd/scalar queues.
    Output is staged and shipped as bf16 (host casts to fp32).
  - full-array (K=128, N=512) warmup matmuls on a memset tile flip HAM
    to K=8/8 before the prologue (partial-array matmuls don't register:
    4.8us of K=65/N=128 never flipped it), and the DMA order puts wqk
    pair0 kc0 first so the first projection matmul starts ~4us earlier.
"""

import math
import os
from collections import deque

import numpy as np
import ml_dtypes

import concourse.bass as bass
import concourse.tile as tile
from concourse import bacc, mybir
from concourse.bass_utils import run_bass_kernel_spmd
from concourse.tile_rust import add_dep_helper

B, S, D, H, HD = 8, 1024, 768, 12, 64
SCALE = 1.0 / math.sqrt(D)
FP = mybir.dt.float32
BF = mybir.dt.bfloat16
KC = D // 128   # 6 contraction chunks
SC = S // 128   # 8 seq chunks
NQ = S // 512   # 2 free-dim chunks of 512
NP = H // 2     # 6 head pairs
LAG = 2         # pv rides LAG slots behind scores

AluOp = mybir.AluOpType
ActFn = mybir.ActivationFunctionType

# Results of the last hardware run (for test harness introspection).
last_results = None


def _build_kernel_body(tc, out_d, xt_d, wqkp_d, wvp_d, wop_d, bqk_d, bo2_d):
    nc = tc.nc

    # Chain every TensorE instruction to the previous one with a no-sync
    # ordering edge so the Tile scheduler preserves the deliberate
    # scores/pv/filler interleave on the in-order PE stream.
    _pe_last = [None]

    def _chain(inst):
        if _pe_last[0] is not None:
            add_dep_helper(inst.ins, _pe_last[0].ins, sync=False,
                           reason="pe-order")
        _pe_last[0] = inst
        return inst

    def MM(*a, reuse_w=False, **k):
        inst = nc.tensor.matmul(*a, **k)
        if reuse_w:
            inst.ins.ldweights = False
        return _chain(inst)

    from contextlib import ExitStack

    with ExitStack() as ctx:
        wpool = ctx.enter_context(tc.tile_pool(name="weights", bufs=1))
        bigs = ctx.enter_context(tc.tile_pool(name="bigs", bufs=1))
        workp = ctx.enter_context(tc.tile_pool(name="work", bufs=1))
        outp = ctx.enter_context(tc.tile_pool(name="outstage", bufs=4))
        accp = ctx.enter_context(tc.tile_pool(name="outacc", bufs=1))
        etp = ctx.enter_context(tc.tile_pool(name="et", bufs=4))
        scp = ctx.enter_context(tc.tile_pool(name="ps_t", bufs=2, space="PSUM"))
        pvp = ctx.enter_context(tc.tile_pool(name="ps_pv", bufs=2, space="PSUM"))

        # ---- persistent sbuf tensors ----
        xT = bigs.tile([128, KC, S], BF)
        wqk_sb = [wpool.tile([128, KC, 256], BF, name=f"wqk_{t}")
                  for t in range(NP)]
        wv_sb = wpool.tile([128, KC, D], BF)
        wo_sb = wpool.tile([128, KC, D], BF)
        bqk_sb = wpool.tile([128, 2 * KC], FP)
        bo_sb = wpool.tile([128, D], FP)
        qkT = bigs.tile([128, 2 * KC, S], BF)
        vaug = bigs.tile([128, SC, H * (HD + 1)], BF)
        vaug4 = vaug.rearrange("p s (h c) -> p s h c", c=HD + 1)
        outT = bigs.tile([128, KC, S], BF)
        wu = wpool.tile([128, 512], BF, name="wu")
        # denominator-broadcast selector: row 64 set, rows 0..63 zero, so
        # ones65.T @ us[0:65] replicates us's row 64 (the softmax
        # denominator staged by fin_a) across a head's 64 bcast outputs.
        ones65 = wpool.tile([65, 64], BF)

        nc.gpsimd.memset(wu[:], 0.0)
        nc.gpsimd.memset(ones65[:], 0.0)
        nc.gpsimd.memset(ones65[64:65, :], 1.0)
        nc.gpsimd.memset(vaug4[:, :, :, HD:HD + 1], 1.0)

        # ---- DMAs, need-ordered ----
        # The sync queue carries only the critical path (wqk pair0 first,
        # then xT in kc order, then wv for the early v fillers); bulk
        # weights ride the otherwise-idle gpsimd queue.
        def dma_xt(kc, nsplit, eng):
            step = 128 // nsplit
            for i in range(nsplit):
                p0, p1 = i * step, (i + 1) * step
                eng.dma_start(xT[p0:p1, kc, :],
                              xt_d[kc * 128 + p0:kc * 128 + p1, :])

        def dma_wqk(t, nsplit, eng):
            w2 = wqkp_d[t * 128:(t + 1) * 128, :].rearrange(
                "p (kc f) -> p kc f", kc=KC)
            step = 128 // nsplit
            for i in range(nsplit):
                p0, p1 = i * step, (i + 1) * step
                eng.dma_start(wqk_sb[t][p0:p1, :, :], w2[p0:p1, :, :])

        # wqk pair0 kc0 (65KB, gates the very first matmul), then xT kc0,
        # then the rest of wqk0, then the xT kc stream the prologue
        # consumes in order; xT triggers alternate between the sync and
        # scalar queues (scalar is idle until the first exp at ~24us) so
        # trigger issue isn't serialized at ~600ns each on sync alone.
        # Bulk weights follow strictly AFTER xT so they never compete
        # with it for HBM queue bandwidth.
        w0 = wqkp_d[0:128, :].rearrange("p (kc f) -> p kc f", kc=KC)
        nc.sync.dma_start(wqk_sb[0][:, 0, :], w0[:, 0, :])
        dma_xt(0, 4, nc.sync)
        nc.sync.dma_start(wqk_sb[0][:, 1:KC, :], w0[:, 1:KC, :])
        for kc in range(1, KC):
            dma_xt(kc, 2, nc.sync)
        bqk2 = bqk_d.rearrange("(p j) -> p j", p=128)
        for i in range(2):
            p0, p1 = i * 64, (i + 1) * 64
            nc.sync.dma_start(bqk_sb[p0:p1, :], bqk2[p0:p1, :])
        for half in range(2):
            p0, p1 = half * 64, (half + 1) * 64
            nc.sync.dma_start(wv_sb[p0:p1, :, :],
                              wvp_d[p0:p1, :].rearrange("p (kc f) -> p kc f",
                                                        kc=KC))
        for t in range(1, NP):
            dma_wqk(t, 2, nc.sync)
        nc.sync.dma_start(wo_sb[:],
                          wop_d.rearrange("p (kc f) -> p kc f", kc=KC))
        nc.sync.dma_start(
            bo_sb[:],
            bo2_d.rearrange("(a f) -> a f", a=1).partition_broadcast(128),
        )

        # ---- building blocks ----
        def qk_half(t, m, n):
            """Project one n-half (512 queries) of one qk m-block (m:
            0..5 = q of pair m, 6..11 = k of pair m-6).  Halving keeps
            each filler at 6 matmuls + one cheap bias-add so the exp
            stream never starves behind a 12-MM blob, while the psum
            lifecycle still completes inside the filler (no "sc"
            rotation deadlock)."""
            pair = t
            qk = 0 if m < KC else 1
            col0 = qk * 128
            ps = scp.tile([128, S], FP, tag="sc", name=f"qk_{m}_{n}")
            for kc in range(KC):
                MM(
                    ps[:, 0:512],
                    wqk_sb[pair][:, kc, col0:col0 + 128],
                    xT[:, kc, n * 512:(n + 1) * 512],
                    start=(kc == 0),
                    stop=(kc == KC - 1),
                )
            nc.vector.tensor_scalar_add(qkT[:, m, n * 512:(n + 1) * 512],
                                        ps[:, 0:512], bqk_sb[:, m:m + 1])

        def qk_prologue():
            """Pair 0 q+k projections, kc-interleaved so both finish ~4
            matmuls after the last xT chunk lands, plus the HAM warmup:
            full-array (K=128, N=512) matmuls on the memset wu tile
            (ready ~7.8us, long before any DMA input) flip the PE clock
            gate to K=8/8 before the real prologue.  Partial-array
            matmuls don't register enough activity - measured: 4.8us of
            K=65/N=128 never flipped.  Warmups write the q psum tile;
            the first real matmul's start=True resets it."""
            psq = scp.tile([128, S], FP, tag="sc", name="qk_pro_q")
            psk = scp.tile([128, S], FP, tag="sc", name="qk_pro_k")
            for i in range(12):
                MM(psq[:, 0:512], wu[:, 0:128], wu[:],
                   start=True, stop=True, reuse_w=(i > 0))
            for kc in range(KC):
                for ps_, col0 in ((psq, 0), (psk, 128)):
                    lhsT = wqk_sb[0][:, kc, col0:col0 + 128]
                    for n in range(NQ):
                        MM(
                            ps_[:, n * 512:(n + 1) * 512],
                            lhsT,
                            xT[:, kc, n * 512:(n + 1) * 512],
                            start=(kc == 0),
                            stop=(kc == KC - 1),
                            reuse_w=(n > 0),
                        )
            nc.vector.tensor_scalar_add(qkT[:, 0, :], psq[:],
                                        bqk_sb[:, 0:1])
            nc.vector.tensor_scalar_add(qkT[:, KC, :], psk[:],
                                        bqk_sb[:, KC:KC + 1])

        def v_pieces(sc):
            """Yield 2 head-column pieces of one v chunk: heads 0-7
            (needed by pv from pair 0) then heads 8-11 (pair 4+)."""
            def piece(half):
                w = 512 if half == 0 else 256
                ps = scp.tile([128, S], FP, tag="sc", name=f"v_{sc}_{half}")
                for kc in range(KC):
                    MM(ps[:, 0:w],
                       xT[:, kc, sc * 128:(sc + 1) * 128],
                       wv_sb[:, kc, half * 512:half * 512 + w],
                       start=(kc == 0), stop=(kc == KC - 1))
                nc.vector.tensor_copy(
                    vaug4[:, sc, half * 8:half * 8 + w // HD, 0:HD],
                    ps[:, 0:w].rearrange("p (h c) -> p h c", c=HD),
                )

            yield lambda: piece(0)
            yield lambda: piece(1)

        def scores_pair(t, sk, et_pair):
            # Both heads' scores matmuls back-to-back: h0 occupies PE row
            # groups 0-1, h1 row groups 2-3 (K=64 each), so each n-block
            # pair streams CONCURRENTLY through separate xbuses.
            ps = [scp.tile([128, S], FP, tag="sc", name=f"sc_{t}_{sk}_{h}")
                  for h in range(2)]
            for n in range(NQ):
                for h01 in range(2):
                    lo, hi = h01 * 64, (h01 + 1) * 64
                    MM(
                        ps[h01][:, n * 512:(n + 1) * 512],
                        qkT[lo:hi, KC + t, sk * 128:(sk + 1) * 128],
                        qkT[lo:hi, t, n * 512:(n + 1) * 512],
                        start=True,
                        stop=True,
                        tile_position=(h01 * 64, 0),
                        reuse_w=(n > 0),
                    )
            for h01 in range(2):
                nc.scalar.activation(
                    et_pair[h01][:, sk, :], ps[h01][:], ActFn.Exp, scale=SCALE
                )

        def pv_chunk(t, sk, et_pair, pv_pair):
            for h01 in range(2):
                h = 2 * t + h01
                for n in range(NQ):
                    MM(
                        pv_pair[h01][:, n * 512:(n + 1) * 512],
                        vaug4[:, sk, h, :],
                        et_pair[h01][:, sk, n * 512:(n + 1) * 512],
                        start=(sk == 0),
                        stop=(sk == SC - 1),
                        reuse_w=(n > 0),
                    )

        def pv_finalize_a(t, pv_pair):
            # Stage each head's full [65, S] accumulator (64 v-rows + the
            # denominator row) to SBUF in ONE copy per head: the pv psum
            # pair is released as early as possible for pair t+1.
            us = []
            for h01 in range(2):
                u = workp.tile([65, S], BF, tag=f"u2{h01}", name=f"u_{t}_{h01}")
                nc.vector.tensor_copy(u[:], pv_pair[h01][:])
                us.append(u)
            return us

        def pv_finalize_b1(t, us):
            # Broadcast each head's staged denominator row across 64
            # psum partitions with the ones65 selector matmul, then a
            # per-head reciprocal PSUM->SBUF into BASE-0 tiles (partition
            # shift is legal when the input is PSUM).  bc_ps is released
            # by the second reciprocal (~2.4us hold) so the scores psum
            # rotation never idles the PE long enough to re-throttle HAM.
            bc_ps = scp.tile([128, S], FP, tag="sc", name=f"bc_{t}")
            for h01 in range(2):
                for n in range(NQ):
                    MM(bc_ps[h01 * 64:(h01 + 1) * 64, n * 512:(n + 1) * 512],
                       ones65[:],
                       us[h01][:, n * 512:(n + 1) * 512],
                       start=True, stop=True,
                       tile_position=(0, h01 * 64),
                       reuse_w=(n > 0))
            # Full-width 128-partition reciprocal into SBUF (the half-width
            # and partition-shifted forms of this custom DVE op corrupt
            # results on real hardware), then stage head h1's lanes to a
            # base-0 tile with a partition-shifted copy FROM PSUM (the one
            # shifted form that is hardware-proven).  bc_ps is released
            # after ~2.4us instead of being held through the multiplies -
            # the longer hold idled the PE >3.4us per pair and re-throttled
            # HAM to half clock.
            rcsb = workp.tile([128, S], FP, tag="rcsb", name=f"rcsb_{t}")
            nc.vector.reciprocal_approx_fast(rcsb[:], bc_ps[:])
            rc1 = workp.tile([HD, S], FP, tag="rc1", name=f"rc1_{t}")
            nc.vector.tensor_copy(rc1[:], rcsb[64:64 + HD, :])
            return rcsb, rc1

        def pv_finalize_b2(t, us, rcs):
            # Deferred past fin_b1; both TT inputs are SBUF at base 0.
            rcsb, rc1 = rcs
            nc.vector.tensor_tensor(
                outT[0:64, t, :], us[0][0:HD, :], rcsb[0:HD, :],
                op=AluOp.mult)
            nc.vector.tensor_tensor(
                outT[64:128, t, :], us[1][0:HD, :], rc1[:],
                op=AluOp.mult)

        # ---- out-projection halves ----
        acc = [accp.tile([128, D], BF, tag=f"acc{sc}", name=f"acc_{sc}")
               for sc in range(SC)]

        def out_half1(sc):
            # kc 0..3 (pairs 0-3) of the output projection for one
            # 128-query block, staged to SBUF with the bias folded in.
            # Runs as late-loop filler once fin_b(3) has written outT.
            ps = scp.tile([128, S], FP, tag="sc", name=f"oA_{sc}")
            for kc in range(4):
                lhsT = outT[:, kc, sc * 128:(sc + 1) * 128]
                MM(ps[:, 0:512], lhsT, wo_sb[:, kc, 0:512],
                   start=(kc == 0), stop=(kc == 3))
                MM(ps[:, 512:D], lhsT, wo_sb[:, kc, 512:D],
                   start=(kc == 0), stop=(kc == 3), reuse_w=True)
            nc.vector.tensor_tensor(acc[sc][:], ps[:, 0:D], bo_sb[:],
                                    op=AluOp.add)

        def drain_warmup():
            # Keep the PE busy (and HAM at K=8/8) through the last pair's
            # finalize chain, which is pure DVE; released by an instant
            # DVE memset.
            ps = scp.tile([128, S], FP, tag="sc", name="wu_drain")
            for i in range(16):
                MM(ps[:, 0:512], wu[:, 0:128], wu[:],
                   start=True, stop=True, reuse_w=(i > 0))
            nc.vector.memset(ps[0:1, 0:1], 0.0)

        def out_half2(sc):
            ps = scp.tile([128, S], FP, tag="sc", name=f"oB_{sc}")
            for kc in range(4, KC):
                lhsT = outT[:, kc, sc * 128:(sc + 1) * 128]
                MM(ps[:, 0:512], lhsT, wo_sb[:, kc, 0:512],
                   start=(kc == 4), stop=(kc == KC - 1))
                MM(ps[:, 512:D], lhsT, wo_sb[:, kc, 512:D],
                   start=(kc == 4), stop=(kc == KC - 1), reuse_w=True)
            osb = outp.tile([128, D], BF, tag="osb", name=f"osb_{sc}")
            nc.vector.tensor_tensor(osb[:], ps[:, 0:D], acc[sc][:],
                                    op=AluOp.add)
            # spread the output DMA triggers across otherwise-idle engine
            # queues (4-way split per block) so the tail isn't serialized
            # on sync's ~600ns/trigger or one queue's descriptor rate
            engs = [nc.sync, nc.gpsimd, nc.scalar]
            for i in range(2):
                p0, p1 = i * 64, (i + 1) * 64
                engs[(2 * sc + i) % 3].dma_start(
                    out_d[sc * 128 + p0:sc * 128 + p1, :], osb[p0:p1, :])

        # ---- prologue: pair 0 q/k projections (stream behind xT DMA) ----
        qk_prologue()

        # ---- main pipeline ----
        # Flat pipeline over 48 (pair, sk) chunks: pv(j-LAG) rides LAG
        # slots behind scores(j); v chunks (pair 0) and next-pair q/k
        # projections are emitted as <=6-MM filler pieces between the
        # scores/pv chunks of each slot.
        et_tiles = {}
        pv_tiles = {}
        fin_keep = {}
        filler = []  # (ready_slot, fn) in FIFO order per ready time

        def run_filler(j):
            for i, (rdy, fn) in enumerate(filler):
                if rdy <= j:
                    filler.pop(i)
                    fn()
                    return

        def emit_pv(j):
            t, sk = j // SC, j % SC
            if sk == 0:
                pv_tiles[t] = [
                    pvp.tile([HD + 1, S], FP, tag="pv", name=f"pv_{2 * t + i}")
                    for i in range(2)
                ]
            pv_chunk(t, sk, et_tiles[t], pv_tiles[t])
            if sk == SC - 1:
                us = pv_finalize_a(t, pv_tiles[t])
                cur = (t + 1) * SC + LAG - 2
                if t == NP - 1:
                    bc = pv_finalize_b1(t, us)
                    pv_finalize_b2(t, us, bc)
                else:
                    def fb1(t=t, us=us):
                        bc = pv_finalize_b1(t, us)
                        filler.append(
                            (0, lambda: pv_finalize_b2(t, us, bc)))
                    filler.append((cur + 5, fb1))
                del pv_tiles[t], et_tiles[t]

        NCH = NP * SC
        for j in range(NCH):
            t, sk = j // SC, j % SC
            if sk == 0:
                et_tiles[t] = [
                    etp.tile([128, SC, S], BF, tag="et", name=f"et_{t}_{i}")
                    for i in range(2)
                ]
            if t == 0:
                # v heads 0-7 feed pv(0, sk) at slot sk+LAG; heads 8-11
                # are first read by pv at pair 4 - place them in the
                # qk-free slots (sk 0,5,6,7) of pairs 1-2 so no slot
                # carries two 1.3us filler pieces
                p1, p2 = v_pieces(sk)
                filler.append((j, p1))
                filler.append((9 + 2 * sk, p2))
            if t + 1 < NP:
                # next pair's q/k projections as four 6-MM halves,
                # alternating with filler-free slots (sk 0,2,4,6) so the
                # exp stream can catch up between heavy slots and the
                # pair-boundary never idles the PE near the 3.4us HAM
                # re-throttle window.  Pair 0 keeps sk 1..4 (wqk1's DMA
                # lands just before its slot-0).
                sks = [1, 2, 3, 4] if t == 0 else [0, 2, 4, 6]
                for i, (mm_, nn) in enumerate(
                        [(t + 1, 0), (t + 1, 1),
                         (KC + t + 1, 0), (KC + t + 1, 1)]):
                    if sk == sks[i]:
                        filler.append(
                            (j, lambda t=t, m=mm_, n=nn: qk_half(t + 1, m, n)))
            if t == NP - 2 and sk == 6:
                # first-half output projection: ready once fin_b(3) (the
                # late-loop filler at slot 37) has written outT[:, 0:4].
                # The last two blocks are held for the drain (slots 48-49)
                # where they replace pure warmup matmuls with real work
                # while the last pair's finalize chain runs on DVE.
                for sc in range(SC):
                    filler.append((38 + sc, lambda sc=sc: out_half1(sc)))
            # Slot order scores -> pv -> fillers: pv allocates no "sc"
            # tile, so it provides ~0.9us of allocation-free PE work that
            # covers the exp's hold on the psum buffer a filler (or the
            # next slot's scores) would otherwise stall on.
            scores_pair(t, sk, et_tiles[t])
            if j >= LAG:
                emit_pv(j - LAG)
            run_filler(j)
            run_filler(j)
        for j in range(NCH, NCH + LAG):
            emit_pv(j - LAG)
            while [f for f in filler if f[0] <= j]:
                run_filler(j)
        while filler:
            run_filler(10 ** 9)

        # ---- output projection second half (kc 4-5, needs pair 4+5) ----
        drain_warmup()
        for sc in range(SC):
            out_half2(sc)


def build():
    """Build + compile the per-core Bass module. Returns the Bacc object."""
    nc = bacc.Bacc("TRN2", target_bir_lowering=False, debug=False, num_devices=B)
    xt_d = nc.dram_tensor("xt", [D, S], BF, kind="ExternalInput").ap()
    wqkp_d = nc.dram_tensor("wqkp", [NP * 128, KC * 256], BF,
                            kind="ExternalInput").ap()
    wvp_d = nc.dram_tensor("wvp", [128, KC * D], BF, kind="ExternalInput").ap()
    wop_d = nc.dram_tensor("wop", [128, KC * D], BF, kind="ExternalInput").ap()
    bqk_d = nc.dram_tensor("bqk", [2 * D], FP, kind="ExternalInput").ap()
    bo2_d = nc.dram_tensor("bo2", [D], FP, kind="ExternalInput").ap()
    out_d = nc.dram_tensor("out", [S, D], BF, kind="ExternalOutput").ap()
    with tile.TileContext(nc) as tc:
        _build_kernel_body(tc, out_d, xt_d, wqkp_d, wvp_d, wop_d, bqk_d, bo2_d)
    nc.compile()
    return nc


def prep_weights(Wqkv, bqkv, Wo, bo):
    """Host-side weight packing (numpy only)."""
    bf16 = ml_dtypes.bfloat16
    # Wqkv [H, D, 3*HD] -> Wq_all/Wk_all/Wv_all [D, H*HD]
    Wq = np.transpose(Wqkv[:, :, 0:HD], (1, 0, 2)).reshape(D, D)
    Wk = np.transpose(Wqkv[:, :, HD:2 * HD], (1, 0, 2)).reshape(D, D)
    Wv = np.transpose(Wqkv[:, :, 2 * HD:], (1, 0, 2)).reshape(D, D)
    # pair-major qk blocks: wqkp[t] = [128, KC, 256] with row p holding
    # W rows {kc*128+p} for all kc, cols = [q pair cols | k pair cols]
    wqkp = np.empty((NP, 128, KC, 256), dtype=bf16)
    for t in range(NP):
        blk = np.concatenate(
            [Wq[:, t * 128:(t + 1) * 128], Wk[:, t * 128:(t + 1) * 128]],
            axis=1,
        )  # [D, 256]
        wqkp[t] = blk.reshape(KC, 128, 256).transpose(1, 0, 2).astype(bf16)
    wqkp = wqkp.reshape(NP * 128, KC * 256)
    # per-partition-contiguous v / o weights: row p = [W[kc*128+p, :] for kc]
    wvp = Wv.reshape(KC, 128, D).transpose(1, 0, 2).reshape(128, KC * D)
    wop = Wo.reshape(KC, 128, D).transpose(1, 0, 2).reshape(128, KC * D)
    # biases: q then k, partition-major [p, j] with j = m-block id
    bq = bqkv[:, 0:HD].reshape(D)
    bk = bqkv[:, HD:2 * HD].reshape(D)
    bv = bqkv[:, 2 * HD:].reshape(D)
    bqk = np.concatenate([bq, bk]).reshape(2 * KC, 128).T  # [128, 12]
    bo2 = bo.astype(np.float64) + bv.astype(np.float64) @ Wo.astype(np.float64)
    return {
        "wqkp": np.ascontiguousarray(wqkp),
        "wvp": np.ascontiguousarray(wvp.astype(bf16)),
        "wop": np.ascontiguousarray(wop.astype(bf16)),
        "bqk": np.ascontiguousarray(bqk.reshape(2 * D).astype(np.float32)),
        "bo2": np.ascontiguousarray(bo2.astype(np.float32)),
    }


_nc_cache = None


def kernel(x, Wqkv, bqkv, Wo, bo):
    global _nc_cache, last_results
    if _nc_cache is None:
        _nc_cache = build()
    nc = _nc_cache
    w = prep_weights(np.asarray(Wqkv), np.asarray(bqkv), np.asarray(Wo),
                     np.asarray(bo))
    bf16 = ml_dtypes.bfloat16
    x = np.asarray(x, dtype=np.float32)
    in_maps = [
        {"xt": np.ascontiguousarray(x[i].T.astype(bf16)), **w}
        for i in range(B)
    ]
    res = run_bass_kernel_spmd(
        nc, in_maps, core_ids=list(range(B)),
        trace=bool(os.environ.get("KERNEL_TRACE")),
    )
    last_results = res
    out = np.stack([res.results[i]["out"] for i in range(B)], axis=0)
    return out.astype(np.float32)


# revision 68
# speedup vs baseline: 1.0161x; 1.0161x over previous
"""Trainium2 Bass kernel for nn_Attention: per-head QKV attention + out-proj.

Contract: kernel(**inputs) takes FULL unsharded inputs
  x [8, 1024, 768] f32, Wqkv [12, 768, 192] f32, bqkv [12, 192] f32,
  Wo [768, 768] f32, bo [768] f32
returns FULL output [8, 1024, 768] f32.

Strategy: pure data-parallel over batch (8 batches -> 8 NeuronCores), no
collectives.  Each core computes its batch end-to-end in bf16 matmuls.

v3 changes vs v2 (HW ~236us, rel err 3.7e-3):
  - the two heads of a pair issue their scores matmuls back-to-back into
    DISJOINT PE row groups (K=64 each, tile_position (0,0)/(64,0)), so
    they stream concurrently: scores PE time halves (~20us).
  - PSUM locked to exactly 8 banks: "sc" slot 2x[128,1024] (scores + all
    transient matmul outputs) and "pv" slot 2x[65,1024] (accumulators).
    The "sc" slot is a 2-deep ring: EVERY tile allocated from it must
    complete its whole read-write lifecycle within 2 subsequent
    allocations or the in-order PE stream stalls (and a >3.4us stall
    re-throttles HAM to half clock for ~7us - the main failure mode).
  - softmax finalize: one [65,S] copy per head stages v-rows +
    denominator and releases pv psum early; ones65-selector matmuls
    broadcast each denominator row; full-width 128-partition
    reciprocal_approx_fast PSUM->SBUF (half-width or partition-shifted
    forms of this custom op corrupt results on real HW); h1's lanes
    reach base partition 0 via a partition-shifted SBUF copy; both
    normalize multiplies are then base-0 SBUF x SBUF.  bc_ps is held
    only ~2.4us, so the ring never stalls the PE into a HAM window.
  - qk projections for the next pair are four 6-MM n-halves placed in
    ALTERNATING slots (sk 0,2,4,6) so heavy and light slots interleave
    and the exp stream never starves; v piece2 fillers fall in the odd
    slots of pairs 1-2.
  - output projection split kc 0-3 / 4-5: the first half runs as
    late-loop fillers (pairs 4-5) with bias folded into bf16 partials;
    only kc 4-5 + the staged add remain after the last pair, with 16
    warmup matmuls keeping HAM warm through the finalize chain and the
    output DMA triggers spread across sync/gpsimd/scalar queues.
    Output is staged and shipped as bf16 (host casts to fp32).
  - full-array (K=128, N=512) warmup matmuls on a memset tile flip HAM
    to K=8/8 before the prologue (partial-array matmuls don't register:
    4.8us of K=65/N=128 never flipped it), and the DMA order puts wqk
    pair0 kc0 first so the first projection matmul starts ~4us earlier.
"""

import math
import os
from collections import deque

import numpy as np
import ml_dtypes

import concourse.bass as bass
import concourse.tile as tile
from concourse import bacc, mybir
from concourse.bass_utils import run_bass_kernel_spmd
from concourse.tile_rust import add_dep_helper

B, S, D, H, HD = 8, 1024, 768, 12, 64
SCALE = 1.0 / math.sqrt(D)
FP = mybir.dt.float32
BF = mybir.dt.bfloat16
KC = D // 128   # 6 contraction chunks
SC = S // 128   # 8 seq chunks
NQ = S // 512   # 2 free-dim chunks of 512
NP = H // 2     # 6 head pairs
LAG = 2         # pv rides LAG slots behind scores

AluOp = mybir.AluOpType
ActFn = mybir.ActivationFunctionType

# Results of the last hardware run (for test harness introspection).
last_results = None


def _build_kernel_body(tc, out_d, xt_d, wqkp_d, wvp_d, wop_d, bqk_d, bo2_d):
    nc = tc.nc

    # Chain every TensorE instruction to the previous one with a no-sync
    # ordering edge so the Tile scheduler preserves the deliberate
    # scores/pv/filler interleave on the in-order PE stream.
    _pe_last = [None]

    def _chain(inst):
        if _pe_last[0] is not None:
            add_dep_helper(inst.ins, _pe_last[0].ins, sync=False,
                           reason="pe-order")
        _pe_last[0] = inst
        return inst

    def MM(*a, reuse_w=False, **k):
        inst = nc.tensor.matmul(*a, **k)
        if reuse_w:
            inst.ins.ldweights = False
        return _chain(inst)

    from contextlib import ExitStack

    with ExitStack() as ctx:
        wpool = ctx.enter_context(tc.tile_pool(name="weights", bufs=1))
        bigs = ctx.enter_context(tc.tile_pool(name="bigs", bufs=1))
        workp = ctx.enter_context(tc.tile_pool(name="work", bufs=1))
        outp = ctx.enter_context(tc.tile_pool(name="outstage", bufs=4))
        accp = ctx.enter_context(tc.tile_pool(name="outacc", bufs=1))
        etp = ctx.enter_context(tc.tile_pool(name="et", bufs=4))
        scp = ctx.enter_context(tc.tile_pool(name="ps_t", bufs=2, space="PSUM"))
        pvp = ctx.enter_context(tc.tile_pool(name="ps_pv", bufs=2, space="PSUM"))

        # ---- persistent sbuf tensors ----
        xT = bigs.tile([128, KC, S], BF)
        wqk_sb = [wpool.tile([128, KC, 256], BF, name=f"wqk_{t}")
                  for t in range(NP)]
        wv_sb = wpool.tile([128, KC, D], BF)
        wo_sb = wpool.tile([128, KC, D], BF)
        bqk_sb = wpool.tile([128, 2 * KC], FP)
        bo_sb = wpool.tile([128, D], FP)
        qkT = bigs.tile([128, 2 * KC, S], BF)
        vaug = bigs.tile([128, SC, H * (HD + 1)], BF)
        vaug4 = vaug.rearrange("p s (h c) -> p s h c", c=HD + 1)
        outT = bigs.tile([128, KC, S], BF)
        wu = wpool.tile([128, 512], BF, name="wu")
        # denominator-broadcast selector: row 64 set, rows 0..63 zero, so
        # ones65.T @ us[0:65] replicates us's row 64 (the softmax
        # denominator staged by fin_a) across a head's 64 bcast outputs.
        ones65 = wpool.tile([65, 64], BF)

        nc.gpsimd.memset(wu[:], 0.0)
        nc.gpsimd.memset(ones65[:], 0.0)
        nc.gpsimd.memset(ones65[64:65, :], 1.0)
        nc.gpsimd.memset(vaug4[:, :, :, HD:HD + 1], 1.0)

        # ---- DMAs, need-ordered, all on the sync trigger queue ----
        def dma_xt(kc, nsplit, eng):
            step = 128 // nsplit
            for i in range(nsplit):
                p0, p1 = i * step, (i + 1) * step
                eng.dma_start(xT[p0:p1, kc, :],
                              xt_d[kc * 128 + p0:kc * 128 + p1, :])

        def dma_wqk(t, nsplit, eng):
            w2 = wqkp_d[t * 128:(t + 1) * 128, :].rearrange(
                "p (kc f) -> p kc f", kc=KC)
            step = 128 // nsplit
            for i in range(nsplit):
                p0, p1 = i * step, (i + 1) * step
                eng.dma_start(wqk_sb[t][p0:p1, :, :], w2[p0:p1, :, :])

        # wqk pair0 kc0 (65KB, gates the very first matmul), then xT kc0,
        # then the rest of wqk0, then the xT kc stream the prologue
        # consumes in order.  Bulk weights follow strictly AFTER xT so
        # they never compete with it for HBM queue bandwidth (triggering
        # them early from other engine queues measured WORSE).
        w0 = wqkp_d[0:128, :].rearrange("p (kc f) -> p kc f", kc=KC)
        nc.sync.dma_start(wqk_sb[0][:, 0, :], w0[:, 0, :])
        dma_xt(0, 4, nc.sync)
        nc.sync.dma_start(wqk_sb[0][:, 1:KC, :], w0[:, 1:KC, :])
        for kc in range(1, KC):
            dma_xt(kc, 2, nc.sync)
        bqk2 = bqk_d.rearrange("(p j) -> p j", p=128)
        for i in range(2):
            p0, p1 = i * 64, (i + 1) * 64
            nc.sync.dma_start(bqk_sb[p0:p1, :], bqk2[p0:p1, :])
        for half in range(2):
            p0, p1 = half * 64, (half + 1) * 64
            nc.sync.dma_start(wv_sb[p0:p1, :, :],
                              wvp_d[p0:p1, :].rearrange("p (kc f) -> p kc f",
                                                        kc=KC))
        for t in range(1, NP):
            dma_wqk(t, 2, nc.sync)
        nc.sync.dma_start(wo_sb[:],
                          wop_d.rearrange("p (kc f) -> p kc f", kc=KC))
        nc.sync.dma_start(
            bo_sb[:],
            bo2_d.rearrange("(a f) -> a f", a=1).partition_broadcast(128),
        )

        # ---- building blocks ----
        def qk_half(t, m, n):
            """Project one n-half (512 queries) of one qk m-block (m:
            0..5 = q of pair m, 6..11 = k of pair m-6).  Halving keeps
            each filler at 6 matmuls + one cheap bias-add so the exp
            stream never starves behind a 12-MM blob, while the psum
            lifecycle still completes inside the filler (no "sc"
            rotation deadlock)."""
            pair = t
            qk = 0 if m < KC else 1
            col0 = qk * 128
            ps = scp.tile([128, S], FP, tag="sc", name=f"qk_{m}_{n}")
            for kc in range(KC):
                MM(
                    ps[:, 0:512],
                    wqk_sb[pair][:, kc, col0:col0 + 128],
                    xT[:, kc, n * 512:(n + 1) * 512],
                    start=(kc == 0),
                    stop=(kc == KC - 1),
                )
            nc.vector.tensor_scalar_add(qkT[:, m, n * 512:(n + 1) * 512],
                                        ps[:, 0:512], bqk_sb[:, m:m + 1])

        def qk_prologue():
            """Pair 0 q+k projections, kc-interleaved so both finish ~4
            matmuls after the last xT chunk lands, plus the HAM warmup:
            full-array (K=128, N=512) matmuls on the memset wu tile
            (ready ~7.8us, long before any DMA input) flip the PE clock
            gate to K=8/8 before the real prologue.  Partial-array
            matmuls don't register enough activity - measured: 4.8us of
            K=65/N=128 never flipped.  Warmups write the q psum tile;
            the first real matmul's start=True resets it."""
            psq = scp.tile([128, S], FP, tag="sc", name="qk_pro_q")
            psk = scp.tile([128, S], FP, tag="sc", name="qk_pro_k")
            for i in range(12):
                MM(psq[:, 0:512], wu[:, 0:128], wu[:],
                   start=True, stop=True, reuse_w=(i > 0))
            for kc in range(KC):
                for ps_, col0 in ((psq, 0), (psk, 128)):
                    lhsT = wqk_sb[0][:, kc, col0:col0 + 128]
                    for n in range(NQ):
                        MM(
                            ps_[:, n * 512:(n + 1) * 512],
                            lhsT,
                            xT[:, kc, n * 512:(n + 1) * 512],
                            start=(kc == 0),
                            stop=(kc == KC - 1),
                            reuse_w=(n > 0),
                        )
            nc.vector.tensor_scalar_add(qkT[:, 0, :], psq[:],
                                        bqk_sb[:, 0:1])
            nc.vector.tensor_scalar_add(qkT[:, KC, :], psk[:],
                                        bqk_sb[:, KC:KC + 1])

        def v_pieces(sc):
            """Yield 2 head-column pieces of one v chunk: heads 0-7
            (needed by pv from pair 0) then heads 8-11 (pair 4+)."""
            def piece(half):
                w = 512 if half == 0 else 256
                ps = scp.tile([128, S], FP, tag="sc", name=f"v_{sc}_{half}")
                for kc in range(KC):
                    MM(ps[:, 0:w],
                       xT[:, kc, sc * 128:(sc + 1) * 128],
                       wv_sb[:, kc, half * 512:half * 512 + w],
                       start=(kc == 0), stop=(kc == KC - 1))
                nc.vector.tensor_copy(
                    vaug4[:, sc, half * 8:half * 8 + w // HD, 0:HD],
                    ps[:, 0:w].rearrange("p (h c) -> p h c", c=HD),
                )

            yield lambda: piece(0)
            yield lambda: piece(1)

        def scores_pair(t, sk, et_pair):
            # Both heads' scores matmuls back-to-back: h0 occupies PE row
            # groups 0-1, h1 row groups 2-3 (K=64 each), so each n-block
            # pair streams CONCURRENTLY through separate xbuses.
            ps = [scp.tile([128, S], FP, tag="sc", name=f"sc_{t}_{sk}_{h}")
                  for h in range(2)]
            for n in range(NQ):
                for h01 in range(2):
                    lo, hi = h01 * 64, (h01 + 1) * 64
                    MM(
                        ps[h01][:, n * 512:(n + 1) * 512],
                        qkT[lo:hi, KC + t, sk * 128:(sk + 1) * 128],
                        qkT[lo:hi, t, n * 512:(n + 1) * 512],
                        start=True,
                        stop=True,
                        tile_position=(h01 * 64, 0),
                        reuse_w=(n > 0),
                    )
            for h01 in range(2):
                nc.scalar.activation(
                    et_pair[h01][:, sk, :], ps[h01][:], ActFn.Exp, scale=SCALE
                )

        def pv_chunk(t, sk, et_pair, pv_pair):
            for h01 in range(2):
                h = 2 * t + h01
                for n in range(NQ):
                    MM(
                        pv_pair[h01][:, n * 512:(n + 1) * 512],
                        vaug4[:, sk, h, :],
                        et_pair[h01][:, sk, n * 512:(n + 1) * 512],
                        start=(sk == 0),
                        stop=(sk == SC - 1),
                        reuse_w=(n > 0),
                    )

        def pv_finalize_a(t, pv_pair):
            # Stage each head's full [65, S] accumulator (64 v-rows + the
            # denominator row) to SBUF in ONE copy per head: the pv psum
            # pair is released as early as possible for pair t+1.
            us = []
            for h01 in range(2):
                u = workp.tile([65, S], BF, tag=f"u2{h01}", name=f"u_{t}_{h01}")
                nc.vector.tensor_copy(u[:], pv_pair[h01][:])
                us.append(u)
            return us

        def pv_finalize_b1(t, us):
            # Broadcast each head's staged denominator row across 64
            # psum partitions with the ones65 selector matmul, then a
            # per-head reciprocal PSUM->SBUF into BASE-0 tiles (partition
            # shift is legal when the input is PSUM).  bc_ps is released
            # by the second reciprocal (~2.4us hold) so the scores psum
            # rotation never idles the PE long enough to re-throttle HAM.
            bc_ps = scp.tile([128, S], FP, tag="sc", name=f"bc_{t}")
            for h01 in range(2):
                for n in range(NQ):
                    MM(bc_ps[h01 * 64:(h01 + 1) * 64, n * 512:(n + 1) * 512],
                       ones65[:],
                       us[h01][:, n * 512:(n + 1) * 512],
                       start=True, stop=True,
                       tile_position=(0, h01 * 64),
                       reuse_w=(n > 0))
            # Full-width 128-partition reciprocal into SBUF (the half-width
            # and partition-shifted forms of this custom DVE op corrupt
            # results on real hardware), then stage head h1's lanes to a
            # base-0 tile with a partition-shifted copy FROM PSUM (the one
            # shifted form that is hardware-proven).  bc_ps is released
            # after ~2.4us instead of being held through the multiplies -
            # the longer hold idled the PE >3.4us per pair and re-throttled
            # HAM to half clock.
            rcsb = workp.tile([128, S], FP, tag="rcsb", name=f"rcsb_{t}")
            nc.vector.reciprocal_approx_fast(rcsb[:], bc_ps[:])
            rc1 = workp.tile([HD, S], FP, tag="rc1", name=f"rc1_{t}")
            nc.vector.tensor_copy(rc1[:], rcsb[64:64 + HD, :])
            return rcsb, rc1

        def pv_finalize_b2(t, us, rcs):
            # Deferred past fin_b1; both TT inputs are SBUF at base 0.
            rcsb, rc1 = rcs
            nc.vector.tensor_tensor(
                outT[0:64, t, :], us[0][0:HD, :], rcsb[0:HD, :],
                op=AluOp.mult)
            nc.vector.tensor_tensor(
                outT[64:128, t, :], us[1][0:HD, :], rc1[:],
                op=AluOp.mult)

        # ---- out-projection halves ----
        acc = [accp.tile([128, D], BF, tag=f"acc{sc}", name=f"acc_{sc}")
               for sc in range(SC)]

        def out_half1(sc):
            # kc 0..3 (pairs 0-3) of the output projection for one
            # 128-query block, staged to SBUF with the bias folded in.
            # Runs as late-loop filler once fin_b(3) has written outT.
            ps = scp.tile([128, S], FP, tag="sc", name=f"oA_{sc}")
            for kc in range(4):
                lhsT = outT[:, kc, sc * 128:(sc + 1) * 128]
                MM(ps[:, 0:512], lhsT, wo_sb[:, kc, 0:512],
                   start=(kc == 0), stop=(kc == 3))
                MM(ps[:, 512:D], lhsT, wo_sb[:, kc, 512:D],
                   start=(kc == 0), stop=(kc == 3), reuse_w=True)
            nc.vector.tensor_tensor(acc[sc][:], ps[:, 0:D], bo_sb[:],
                                    op=AluOp.add)

        def drain_warmup():
            # Keep the PE busy (and HAM at K=8/8) through the last pair's
            # finalize chain, which is pure DVE; released by an instant
            # DVE memset.
            ps = scp.tile([128, S], FP, tag="sc", name="wu_drain")
            for i in range(16):
                MM(ps[:, 0:512], wu[:, 0:128], wu[:],
                   start=True, stop=True, reuse_w=(i > 0))
            nc.vector.memset(ps[0:1, 0:1], 0.0)

        def out_half2(sc):
            ps = scp.tile([128, S], FP, tag="sc", name=f"oB_{sc}")
            for kc in range(4, KC):
                lhsT = outT[:, kc, sc * 128:(sc + 1) * 128]
                MM(ps[:, 0:512], lhsT, wo_sb[:, kc, 0:512],
                   start=(kc == 4), stop=(kc == KC - 1))
                MM(ps[:, 512:D], lhsT, wo_sb[:, kc, 512:D],
                   start=(kc == 4), stop=(kc == KC - 1), reuse_w=True)
            osb = outp.tile([128, D], BF, tag="osb", name=f"osb_{sc}")
            nc.vector.tensor_tensor(osb[:], ps[:, 0:D], acc[sc][:],
                                    op=AluOp.add)
            # spread the output DMA triggers across otherwise-idle engine
            # queues (4-way split per block) so the tail isn't serialized
            # on sync's ~600ns/trigger or one queue's descriptor rate
            engs = [nc.sync, nc.gpsimd, nc.scalar]
            for i in range(2):
                p0, p1 = i * 64, (i + 1) * 64
                engs[(2 * sc + i) % 3].dma_start(
                    out_d[sc * 128 + p0:sc * 128 + p1, :], osb[p0:p1, :])

        # ---- prologue: pair 0 q/k projections (stream behind xT DMA) ----
        qk_prologue()

        # ---- main pipeline ----
        # Flat pipeline over 48 (pair, sk) chunks: pv(j-LAG) rides LAG
        # slots behind scores(j); v chunks (pair 0) and next-pair q/k
        # projections are emitted as <=6-MM filler pieces between the
        # scores/pv chunks of each slot.
        et_tiles = {}
        pv_tiles = {}
        fin_keep = {}
        filler = []  # (ready_slot, fn) in FIFO order per ready time

        def run_filler(j):
            for i, (rdy, fn) in enumerate(filler):
                if rdy <= j:
                    filler.pop(i)
                    fn()
                    return

        def emit_pv(j):
            t, sk = j // SC, j % SC
            if sk == 0:
                pv_tiles[t] = [
                    pvp.tile([HD + 1, S], FP, tag="pv", name=f"pv_{2 * t + i}")
                    for i in range(2)
                ]
            pv_chunk(t, sk, et_tiles[t], pv_tiles[t])
            if sk == SC - 1:
                us = pv_finalize_a(t, pv_tiles[t])
                cur = (t + 1) * SC + LAG - 2
                if t == NP - 1:
                    bc = pv_finalize_b1(t, us)
                    pv_finalize_b2(t, us, bc)
                else:
                    def fb1(t=t, us=us):
                        bc = pv_finalize_b1(t, us)
                        filler.append(
                            (0, lambda: pv_finalize_b2(t, us, bc)))
                    filler.append((cur + 5, fb1))
                del pv_tiles[t], et_tiles[t]

        NCH = NP * SC
        for j in range(NCH):
            t, sk = j // SC, j % SC
            if sk == 0:
                et_tiles[t] = [
                    etp.tile([128, SC, S], BF, tag="et", name=f"et_{t}_{i}")
                    for i in range(2)
                ]
            if t == 0:
                # v heads 0-7 feed pv(0, sk) at slot sk+LAG; heads 8-11
                # are first read by pv at pair 4 - place them in the
                # qk-free slots (sk 0,5,6,7) of pairs 1-2 so no slot
                # carries two 1.3us filler pieces
                p1, p2 = v_pieces(sk)
                filler.append((j, p1))
                filler.append((9 + 2 * sk, p2))
            if t + 1 < NP:
                # next pair's q/k projections as four 6-MM halves,
                # alternating with filler-free slots (sk 0,2,4,6) so the
                # exp stream can catch up between heavy slots and the
                # pair-boundary never idles the PE near the 3.4us HAM
                # re-throttle window.  Pair 0 keeps sk 1..4 (wqk1's DMA
                # lands just before its slot-0).
                sks = [1, 2, 3, 4] if t == 0 else [0, 2, 4, 6]
                for i, (mm_, nn) in enumerate(
                        [(t + 1, 0), (t + 1, 1),
                         (KC + t + 1, 0), (KC + t + 1, 1)]):
                    if sk == sks[i]:
                        filler.append(
                            (j, lambda t=t, m=mm_, n=nn: qk_half(t + 1, m, n)))
            if t == NP - 2 and sk == 6:
                # first-half output projection: ready once fin_b(3) (the
                # late-loop filler at slot 37) has written outT[:, 0:4].
                # The last two blocks are held for the drain (slots 48-49)
                # where they replace pure warmup matmuls with real work
                # while the last pair's finalize chain runs on DVE.
                for sc in range(SC):
                    filler.append((38 + sc, lambda sc=sc: out_half1(sc)))
            # Slot order scores -> pv -> fillers: pv allocates no "sc"
            # tile, so it provides ~0.9us of allocation-free PE work that
            # covers the exp's hold on the psum buffer a filler (or the
            # next slot's scores) would otherwise stall on.
            scores_pair(t, sk, et_tiles[t])
            if j >= LAG:
                emit_pv(j - LAG)
            run_filler(j)
            run_filler(j)
        for j in range(NCH, NCH + LAG):
            emit_pv(j - LAG)
            while [f for f in filler if f[0] <= j]:
                run_filler(j)
        while filler:
            run_filler(10 ** 9)

        # ---- output projection second half (kc 4-5, needs pair 4+5) ----
        drain_warmup()
        for sc in range(SC):
            out_half2(sc)


def build():
    """Build + compile the per-core Bass module. Returns the Bacc object."""
    nc = bacc.Bacc("TRN2", target_bir_lowering=False, debug=False, num_devices=B)
    xt_d = nc.dram_tensor("xt", [D, S], BF, kind="ExternalInput").ap()
    wqkp_d = nc.dram_tensor("wqkp", [NP * 128, KC * 256], BF,
                            kind="ExternalInput").ap()
    wvp_d = nc.dram_tensor("wvp", [128, KC * D], BF, kind="ExternalInput").ap()
    wop_d = nc.dram_tensor("wop", [128, KC * D], BF, kind="ExternalInput").ap()
    bqk_d = nc.dram_tensor("bqk", [2 * D], FP, kind="ExternalInput").ap()
    bo2_d = nc.dram_tensor("bo2", [D], FP, kind="ExternalInput").ap()
    out_d = nc.dram_tensor("out", [S, D], BF, kind="ExternalOutput").ap()
    with tile.TileContext(nc) as tc:
        _build_kernel_body(tc, out_d, xt_d, wqkp_d, wvp_d, wop_d, bqk_d, bo2_d)
    nc.compile()
    return nc


def prep_weights(Wqkv, bqkv, Wo, bo):
    """Host-side weight packing (numpy only)."""
    bf16 = ml_dtypes.bfloat16
    # Wqkv [H, D, 3*HD] -> Wq_all/Wk_all/Wv_all [D, H*HD]
    Wq = np.transpose(Wqkv[:, :, 0:HD], (1, 0, 2)).reshape(D, D)
    Wk = np.transpose(Wqkv[:, :, HD:2 * HD], (1, 0, 2)).reshape(D, D)
    Wv = np.transpose(Wqkv[:, :, 2 * HD:], (1, 0, 2)).reshape(D, D)
    # pair-major qk blocks: wqkp[t] = [128, KC, 256] with row p holding
    # W rows {kc*128+p} for all kc, cols = [q pair cols | k pair cols]
    wqkp = np.empty((NP, 128, KC, 256), dtype=bf16)
    for t in range(NP):
        blk = np.concatenate(
            [Wq[:, t * 128:(t + 1) * 128], Wk[:, t * 128:(t + 1) * 128]],
            axis=1,
        )  # [D, 256]
        wqkp[t] = blk.reshape(KC, 128, 256).transpose(1, 0, 2).astype(bf16)
    wqkp = wqkp.reshape(NP * 128, KC * 256)
    # per-partition-contiguous v / o weights: row p = [W[kc*128+p, :] for kc]
    wvp = Wv.reshape(KC, 128, D).transpose(1, 0, 2).reshape(128, KC * D)
    wop = Wo.reshape(KC, 128, D).transpose(1, 0, 2).reshape(128, KC * D)
    # biases: q then k, partition-major [p, j] with j = m-block id
    bq = bqkv[:, 0:HD].reshape(D)
    bk = bqkv[:, HD:2 * HD].reshape(D)
    bv = bqkv[:, 2 * HD:].reshape(D)
    bqk = np.concatenate([bq, bk]).reshape(2 * KC, 128).T  # [128, 12]
    bo2 = bo.astype(np.float64) + bv.astype(np.float64) @ Wo.astype(np.float64)
    return {
        "wqkp": np.ascontiguousarray(wqkp),
        "wvp": np.ascontiguousarray(wvp.astype(bf16)),
        "wop": np.ascontiguousarray(wop.astype(bf16)),
        "bqk": np.ascontiguousarray(bqk.reshape(2 * D).astype(np.float32)),
        "bo2": np.ascontiguousarray(bo2.astype(np.float32)),
    }


_nc_cache = None


def kernel(x, Wqkv, bqkv, Wo, bo):
    global _nc_cache, last_results
    if _nc_cache is None:
        _nc_cache = build()
    nc = _nc_cache
    w = prep_weights(np.asarray(Wqkv), np.asarray(bqkv), np.asarray(Wo),
                     np.asarray(bo))
    bf16 = ml_dtypes.bfloat16
    x = np.asarray(x, dtype=np.float32)
    in_maps = [
        {"xt": np.ascontiguousarray(x[i].T.astype(bf16)), **w}
        for i in range(B)
    ]
    res = run_bass_kernel_spmd(
        nc, in_maps, core_ids=list(range(B)),
        trace=bool(os.environ.get("KERNEL_TRACE")),
    )
    last_results = res
    out = np.stack([res.results[i]["out"] for i in range(B)], axis=0)
    return out.astype(np.float32)
